# revision 32
# baseline (speedup 1.0000x reference)
"""Distributed Trainium2 kernel for causal multi-head attention with RoPE.

Problem: B=2, S=2048, E=2048, N=16 heads, H=128 head-dim.
Sharding: 8 cores = 2 (batch, data axis) x 4 (head groups, model axis).
Each core:
  phase 1: Q/K/V projections for its 4 heads (bf16 matmuls, f32 accum),
           RoPE applied to q^T/k^T in [H, S] layout.  Q runs first over
           all four 512-col s-chunks, then K+V per sc pair.  Both Q and
           K passes iterate ec-OUTER over head pairs (8 PSUM banks, two
           pair-groups alternating buffers) so each x strip is consumed
           over a quarter of the pair window -- halves the peak HBM
           demand vs head-major chains.  All loads ride the sync (SP)
           hardware-DGE queue (the Pool queue is a software DGE on
           gpsimd and measurably slower on hw), interleaved in
           consumption order: wq chunks between strip pairs, wk only
           after the Q half-1 strips.  cos/sin tables are bf16.
  phase 2: causal attention, transposed-score formulation.  Scores
           L^T[t,s] computed in PAIRS of 128-t blocks into a 2-bank
           PSUM tile; one 1024-wide exp per pair (halves the ACT
           per-instruction PSUM-access bubbles; ACT is the phase-2
           critical engine).  Diagonal blocks (w=512/384/256/128) pack
           as [512|384+128] + [256] so 2 exps/chain replace 4, small
           tile first so the next chain's PSUM WAR clears early.
           Softmax denominators are fully tree-reduced on the DVE
           (bf16 adds) into ONE [128,512] tensor per chain -> a single
           ones-column matmul stream (was sc+1 streams).  Each chain's
           tail (last two AVs, denominator matmul, normalization, a^T
           store) is deferred into the next chain, filling the PE slack
           while the diagonal exps run.  Chains are ordered sc2/sc0/sc1
           interleaved (ACT-heavy next to ACT-light), then sc3 after
           AG1.
  phase 3: two AllGathers per 4-core group split by s-columns; a^T is
           stored p-major ([sc, p, head, col]) so the phase-3 gather
           DMAs read contiguous 4KB runs per partition.  Each core
           computes o for a different 512-wide E-slice.
Host side re-assembles the 8 [2048, 512] E-chunks into [2, 2048, 2048].

PSUM: lt pair-tiles 2x2 banks + av 3 + sm 1 = 8 banks in phase 2;
phase 3 opens its own pso pool after the attention pools close.
PSUM->SBUF copies (v, osb) run on the ACT engine (idle in phases 1/3;
the DVE is strict in-order, so copies there delay rope ops across loop
iterations -- measured ~7% slower on hw).
"""

import contextlib

import numpy as np
import ml_dtypes

import concourse.mybir as mybir
import concourse.tile as tile
from concourse import bacc
from concourse.bass_utils import run_bass_kernel_spmd

B, S, E, N, H = 2, 2048, 2048, 16, 128
P = 128
NCORES = 8
NH_LOC = N // 4          # 4 heads per core
ECHUNK = E // 4          # 512 output-embedding columns per core
EC = E // P              # 16 contraction chunks
ST = S // P              # 16 seq tiles of 128
F32 = mybir.dt.float32
BF16 = mybir.dt.bfloat16

REPLICA_GROUPS = [[0, 1, 2, 3], [4, 5, 6, 7]]

TRACE = False
LAST_RESULTS = None


def _rope_tables():
    """cos^T / sin^T tables [H, S] bf16, sign-folded and scaled by 128**-0.25.

    Matches reference.sine_table computed in f32 (angles formed with f32
    arithmetic, sin/cos evaluated in f64 of the f32 angle).
    """
    fraction = np.arange(0, H, 2, dtype=np.float32) / np.float32(H)
    timescale = np.float32(10000.0) ** fraction
    inv = (np.float32(1.0) / timescale).astype(np.float32)
    ang = (np.arange(S, dtype=np.float32)[:, None] * inv[None, :]).astype(np.float32)
    ang = np.concatenate([ang, ang], axis=1)        # [S, H]
    sin = np.sin(ang.astype(np.float64))
    cos = np.cos(ang.astype(np.float64))
    scale = float(H) ** -0.25
    cosT = (cos.T * scale).astype(ml_dtypes.bfloat16)   # [H, S]
    sinT = (sin.T * scale)
    sinT[:H // 2] *= -1.0                               # rotate_half sign
    sinT = sinT.astype(ml_dtypes.bfloat16)
    return np.ascontiguousarray(cosT), np.ascontiguousarray(sinT)


def _phase1(nc, tc, qT_sc, kT_sc, v_sc, cos_sb, sin_sb, cos_srcs,
            w_sbs, xpre_pool, xqT_r, xkT_r, wq_r, wk_r, wv_r, sfx=""):
    """QKV proj + RoPE -> per-s-chunk qT/kT [h, n, 512] and v [t, tt, nh].

    Pass 1: Q for all four s-chunks (paired, shared W stationary).
    Pass 2: K + V per s-chunk pair (sharing the xk strips).
    All loads go on the sync (SP) hardware-DGE queue, interleaved in
    consumption order.
    """
    wq_sb, wk_sb, wv_sb = w_sbs
    with (
        tc.tile_pool(name="xin" + sfx, bufs=2) as x_pool,
        tc.tile_pool(name="rope_tmp" + sfx, bufs=3) as tmp_pool,
        tc.tile_pool(name="ph1_psum" + sfx, bufs=2, space="PSUM") as pp1,
    ):
        def load_strips(xT_r, scs, pre=False, js=(0, 1, 2, 3)):
            # The first Q strips (j=0) come from the hoisted xpre pool
            # so the next loop iteration can start before this one's
            # phase 3 releases the per-rep strip space.
            st = {}
            for j in js:
                for si, sc in enumerate(scs):
                    cols = slice(sc * 512, (sc + 1) * 512)
                    if pre and j == 0:
                        t = xpre_pool.tile([P, 4, 512], BF16,
                                           tag=f"xp{si}", name=f"xp{si}")
                    else:
                        t = x_pool.tile([P, 4, 512], BF16, tag=f"x{si}{j}")
                    nc.sync.dma_start(t[:], xT_r[:, 4 * j:4 * j + 4, cols])
                    st[(sc, j)] = t
            return st

        # weights in 2-ec chunks so the first matmuls wait on 256KB, not
        # 2MB.  The first two chunks go on sync AHEAD of the strips (the
        # Pool SWDGE is still busy with const-init at t=0); wk is issued
        # only after the Q half-1 strips (it isn't needed until the K
        # pass, and its 2MB would steal HBM slots from strips the PE is
        # about to consume)
        nc.sync.dma_start(wq_sb[:, 0:2, :], wq_r[:, 0:2, :])
        xq_st = load_strips(xqT_r, (0, 1), pre=True, js=(0,))
        nc.sync.dma_start(wq_sb[:, 2:4, :], wq_r[:, 2:4, :])
        xq_st.update(load_strips(xqT_r, (0, 1), js=(1,)))
        for j in range(2, 4):
            nc.sync.dma_start(wq_sb[:, 2 * j:2 * j + 2, :],
                              wq_r[:, 2 * j:2 * j + 2, :])
        xq_st.update(load_strips(xqT_r, (0, 1), js=(2, 3)))
        for j in range(4, 8):
            nc.sync.dma_start(wq_sb[:, 2 * j:2 * j + 2, :],
                              wq_r[:, 2 * j:2 * j + 2, :])
        nc.sync.dma_start(cos_sb[:], cos_srcs[0])
        nc.sync.dma_start(sin_sb[:], cos_srcs[1])

        def rope(ps, dst, cols):
            """dst = ps*cos + shift128(ps)*sin  (bf16 out), cols into S."""
            w = cols.stop - cols.start
            t_sin = tmp_pool.tile([P, 512], BF16, tag="t_sin")
            nc.vector.tensor_tensor(
                out=t_sin[0:64, :w], in0=ps[64:128, :w],
                in1=sin_sb[0:64, cols], op=mybir.AluOpType.mult)
            nc.vector.tensor_tensor(
                out=t_sin[64:128, :w], in0=ps[0:64, :w],
                in1=sin_sb[64:128, cols], op=mybir.AluOpType.mult)
            t_cos = tmp_pool.tile([P, 512], BF16, tag="t_cos")
            nc.vector.tensor_tensor(
                out=t_cos[:, :w], in0=ps[:, :w],
                in1=cos_sb[:, cols], op=mybir.AluOpType.mult)
            nc.vector.tensor_add(out=dst, in0=t_cos[:, :w], in1=t_sin[:, :w])

        def proj_pass(w_sb, st_map, dst_sc, scs, namep):
            # ec-outer over head PAIRS: each x strip is consumed over a
            # quarter of the pair window instead of back-to-back within
            # one head chain, halving the peak DMA demand.  The two
            # pair-groups alternate PSUM buffers (bufs=2) so ropes of
            # one group hide under the other group's matmuls.
            for pg in range(2):
                ns = (2 * pg, 2 * pg + 1)
                ps = {(k, i): pp1.tile([P, 512], F32, tag=f"pp{k}{i}",
                                       name=f"{namep}{pg}{k}{i}")
                      for k in range(2) for i in range(2)}
                for ec in range(EC):
                    for k, n in enumerate(ns):
                        hs = slice(n * H, (n + 1) * H)
                        for i, sc in enumerate(scs):
                            nc.tensor.matmul(
                                ps[(k, i)], w_sb[:, ec, hs],
                                st_map[(sc, ec // 4)][:, ec % 4, :],
                                start=(ec == 0), stop=(ec == EC - 1),
                                skip_group_check=True)
                for k, n in enumerate(ns):
                    for i, sc in enumerate(scs):
                        rope(ps[(k, i)], dst_sc[sc][:, n, :],
                             slice(sc * 512, (sc + 1) * 512))

        # ---------------- pass 1: Q over sc pairs ----------------
        for half in range(2):
            scs = (2 * half, 2 * half + 1)
            if half == 1:
                xq_st.update(load_strips(xqT_r, scs))
                nc.sync.dma_start(wk_sb[:], wk_r)
            proj_pass(wq_sb, xq_st, qT_sc, scs, f"psq{half}")

        # ---------------- pass 2: K + V over sc pairs ----------------
        xk_st = load_strips(xkT_r, (0, 1))
        nc.sync.dma_start(wv_sb[:], wv_r)
        for half in range(2):
            scs = (2 * half, 2 * half + 1)
            if half == 1:
                xk_st.update(load_strips(xkT_r, scs))
            proj_pass(wk_sb, xk_st, kT_sc, scs, f"psk{half}")
            for si, sc in enumerate(scs):
                for tt in range(4):                 # v for 4 t-tiles
                    tsl = slice(tt * P, (tt + 1) * P)
                    psv = pp1.tile([P, 512], F32, tag=f"pp{tt % 2}{si}",
                                   name=f"psv{sc}{tt}")
                    for ec in range(EC):
                        nc.tensor.matmul(
                            psv, xk_st[(sc, ec // 4)][:, ec % 4, tsl],
                            wv_sb[:, ec, :],
                            start=(ec == 0), stop=(ec == EC - 1))
                    nc.scalar.copy(out=v_sc[sc][:, tt, :], in_=psv[:])


def _attn_sc(nc, tc, pools, sc, heads, qT_sc, kT_sc, v_sc, consts, ag_in,
             pending):
    """Causal attention chains for s-chunk sc, given heads.

    Scores/exp run in pairs of t-blocks ([P,2,512] PSUM tiles, one
    1024-wide exp).  The diagonal 4 blocks (w=512/384/256/128) pack as
    [512|384+128] + [256] -> 2 exps.  Denominators accumulate on the
    DVE into one [128,512] tensor -> single ones-column matmul.
    Normalization of each chain is deferred until after the next
    chain's matmuls are issued (pending holds one entry).
    """
    (elt_pool, qsum_pool, rc_pool, bc_pool, at_pool,
     lt_psum, av_psum, sm_psum) = pools
    maskT, ones_col = consts
    EXP = mybir.ActivationFunctionType.Exp

    for n in heads:
        hs = slice(n * H, (n + 1) * H)
        if pending:
            # finish the previous chain (its last AVs, denominator
            # matmul, normalization, a^T store) now: its exps are long
            # done, and this PE work fills the ACT-bound slack while
            # the previous chain's diagonal exps run
            _flush_one(nc, pools, ag_in, pending)
        avp = av_psum.tile([P, 512], F32, tag="av")

        # pv matmuls run TWO blocks behind their exp so the PE computes
        # the next pair's scores while the ACT finishes the exp; the
        # 2-deep queue gives the 1024-wide exp a full pair of slack
        pend_pv = []

        def issue_pv(avp_=avp, pend=pend_pv, last=False):
            vblk, elt_ap, col0_, tb0 = pend.pop(0)
            nc.tensor.matmul(
                avp_[:, col0_:512], vblk, elt_ap,
                start=(tb0 == 0), stop=(last and not pend),
                skip_group_check=True)

        def push_pv(vblk, elt_ap, col0_, tb0, pend=pend_pv,
                    issue=issue_pv):
            pend.append((vblk, elt_ap, col0_, tb0))
            while len(pend) > 2:
                issue()

        qacc = None
        # full t-blocks in pairs: one 2-bank PSUM tile + 1024-wide exp
        for q in range(sc):
            pair_elts = []
            for h2 in range(2):
                lt2 = lt_psum.tile([P, 2, 512], F32, tag="lt")
                for jj in range(2):
                    j = 2 * h2 + jj
                    kblk = kT_sc[q][:, n, j * P:(j + 1) * P]
                    nc.tensor.matmul(lt2[:, jj, :], kblk, qT_sc[sc][:, n, :],
                                     start=True, stop=True)
                elt2 = elt_pool.tile([P, 2, 512], BF16, tag=f"elt{h2}")
                nc.scalar.activation(out=elt2[:], in_=lt2[:], func=EXP)
                for jj in range(2):
                    j = 2 * h2 + jj
                    push_pv(v_sc[q][:, j, hs], elt2[:, jj, :], 0, 4 * q + j)
                pair_elts.append(elt2)
            s01 = qsum_pool.tile([P, 512], BF16, tag="s01")
            nc.vector.tensor_add(out=s01[:], in0=pair_elts[0][:, 0, :],
                                 in1=pair_elts[0][:, 1, :])
            s23 = qsum_pool.tile([P, 512], BF16, tag="s23")
            nc.vector.tensor_add(out=s23[:], in0=pair_elts[1][:, 0, :],
                                 in1=pair_elts[1][:, 1, :])
            if qacc is None:
                qacc = qsum_pool.tile([P, 512], BF16, tag="qacc")
                nc.vector.tensor_add(out=qacc[:], in0=s01[:], in1=s23[:])
            else:
                sq = qsum_pool.tile([P, 512], BF16, tag="sq")
                nc.vector.tensor_add(out=sq[:], in0=s01[:], in1=s23[:])
                nc.vector.tensor_add(out=qacc[:], in0=qacc[:], in1=sq[:])

        # diagonal blocks packed: tileA = [j0: 512 | j1: 384, j3: 128],
        # tileB = [j2: 256]; 2 exps per chain instead of 4
        def kb(j):
            return kT_sc[sc][:, n, j * P:(j + 1) * P]

        # ltB (the small j2 block) goes FIRST so the next chain's lt
        # WAR resolves against its cheap 256-wide exp, not the 1024-wide
        # eA exp
        qn = qT_sc[sc][:, n, :]
        ltB = lt_psum.tile([P, 2, 512], F32, tag="lt")
        nc.tensor.matmul(ltB[:, 0, 0:256], kb(2), qn[:, 256:512],
                         start=True, stop=True)
        eB = elt_pool.tile([P, 2, 512], BF16, tag="elt1")
        nc.scalar.activation(out=eB[:, 0, 0:256], in_=ltB[:, 0, 0:256],
                             func=EXP)
        ltA = lt_psum.tile([P, 2, 512], F32, tag="lt")
        nc.tensor.matmul(ltA[:, 0, :], kb(0), qn, start=True, stop=True)
        nc.tensor.matmul(ltA[:, 1, 0:384], kb(1), qn[:, 128:512],
                         start=True, stop=True)
        nc.tensor.matmul(ltA[:, 1, 384:512], kb(3), qn[:, 384:512],
                         start=True, stop=True)
        eA = elt_pool.tile([P, 2, 512], BF16, tag="elt0")
        nc.scalar.activation(out=eA[:], in_=ltA[:], func=EXP)

        # causal masks: the first 128 valid cols of each diagonal block
        nc.vector.tensor_mul(out=eA[:, 0, 0:P], in0=eA[:, 0, 0:P],
                             in1=maskT[:])
        nc.vector.tensor_mul(out=eA[:, 1, 0:P], in0=eA[:, 1, 0:P],
                             in1=maskT[:])
        nc.vector.tensor_mul(out=eA[:, 1, 384:512], in0=eA[:, 1, 384:512],
                             in1=maskT[:])
        nc.vector.tensor_mul(out=eB[:, 0, 0:P], in0=eB[:, 0, 0:P],
                             in1=maskT[:])

        push_pv(v_sc[sc][:, 0, hs], eA[:, 0, :], 0, 4 * sc + 0)
        push_pv(v_sc[sc][:, 1, hs], eA[:, 1, 0:384], 128, 4 * sc + 1)

        # denominator: one DVE-accumulated [128,512] -> single matmul
        # (the matmul itself is deferred into the next chain)
        ds = qsum_pool.tile([P, 512], BF16, tag="ds")
        nc.vector.tensor_copy(out=ds[:], in_=eA[:, 0, :])
        nc.vector.tensor_add(out=ds[:, 128:512], in0=ds[:, 128:512],
                             in1=eA[:, 1, 0:384])
        nc.vector.tensor_add(out=ds[:, 256:512], in0=ds[:, 256:512],
                             in1=eB[:, 0, 0:256])
        nc.vector.tensor_add(out=ds[:, 384:512], in0=ds[:, 384:512],
                             in1=eA[:, 1, 384:512])
        if qacc is not None:
            nc.vector.tensor_add(out=ds[:], in0=ds[:], in1=qacc[:])

        pend_tail = [
            (v_sc[sc][:, 2, hs], eB[:, 0, 0:256], 256, 4 * sc + 2),
            (v_sc[sc][:, 3, hs], eA[:, 1, 384:512], 384, 4 * sc + 3),
        ]

        def finish(avp_=avp, ds_=ds, pend=pend_pv, tail=pend_tail,
                   n_=n, sc_=sc):
            for vblk, elt_ap, col0_, tb0 in pend + tail[:-1]:
                nc.tensor.matmul(avp_[:, col0_:512], vblk, elt_ap,
                                 start=(tb0 == 0), stop=False,
                                 skip_group_check=True)
            vblk, elt_ap, col0_, tb0 = tail[-1]
            nc.tensor.matmul(avp_[:, col0_:512], vblk, elt_ap,
                             start=False, stop=True, skip_group_check=True)
            smp = sm_psum.tile([1, 512], F32, tag="sm")
            nc.tensor.matmul(smp[:], ones_col[:], ds_[:], start=True,
                             stop=True, skip_group_check=True)
            rc = rc_pool.tile([1, 512], F32, tag="rc")
            nc.vector.reciprocal(rc[:], smp[:])
            bcs = bc_pool.tile([P, 512], F32, tag="bcs")
            nc.gpsimd.partition_broadcast(bcs[:], rc[:])
            at = at_pool.tile([P, 512], BF16, tag="at")
            nc.vector.tensor_mul(out=at[:], in0=avp_[:], in1=bcs[:])
            nc.sync.dma_start(ag_in.ap()[sc_, :, n_, :], at[:])

        pending.append(finish)


def _build(reps=1, with_cc=True, loop_trips=0):
    nc = bacc.Bacc("TRN2", target_bir_lowering=False, debug=False,
                   num_devices=NCORES)

    xqT = nc.dram_tensor("xqT", [E, S], BF16, kind="ExternalInput")
    xkT = nc.dram_tensor("xkT", [E, S], BF16, kind="ExternalInput")
    wq = nc.dram_tensor("wq", [E, NH_LOC * H], BF16, kind="ExternalInput")
    wk = nc.dram_tensor("wk", [E, NH_LOC * H], BF16, kind="ExternalInput")
    wv = nc.dram_tensor("wv", [E, NH_LOC * H], BF16, kind="ExternalInput")
    wo = nc.dram_tensor("wo", [N * H, ECHUNK], BF16, kind="ExternalInput")
    cosT = nc.dram_tensor("cosT", [H, S], BF16, kind="ExternalInput")
    sinT = nc.dram_tensor("sinT", [H, S], BF16, kind="ExternalInput")
    out = nc.dram_tensor("out", [S, ECHUNK], F32, kind="ExternalOutput")

    # p-major so the phase-3 gather DMAs read contiguous 4KB runs
    ag_in = nc.dram_tensor("ag_in", [4, P, NH_LOC, 512], BF16)
    ag_out1 = nc.dram_tensor("ag_out1", [4, 3, P, NH_LOC, 512], BF16)
    ag_out2 = nc.dram_tensor("ag_out2", [4, P, NH_LOC, 512], BF16)

    xqT_r = xqT.ap().rearrange("(eo p) s -> p eo s", p=P)    # [128, 16, 2048]
    xkT_r = xkT.ap().rearrange("(eo p) s -> p eo s", p=P)
    wq_r = wq.ap().rearrange("(eo p) m -> p eo m", p=P)      # [128, 16, 512]
    wk_r = wk.ap().rearrange("(eo p) m -> p eo m", p=P)
    wv_r = wv.ap().rearrange("(eo p) m -> p eo m", p=P)
    wo_r = wo.ap().rearrange("(n p) e -> p n e", p=P)        # [128, 16, 512]

    with tile.TileContext(nc) as tc:
        with tc.tile_pool(name="const", bufs=1) as const_pool:
            # multiplicative causal mask for L^T diagonal blocks:
            # keep where s - t >= 0
            maskT = const_pool.tile([P, P], BF16)
            nc.gpsimd.memset(maskT, 1.0)
            nc.gpsimd.affine_select(
                out=maskT, in_=maskT,
                compare_op=mybir.AluOpType.is_ge, fill=0.0,
                base=0, pattern=[[1, P]], channel_multiplier=-1)
            ones_col = const_pool.tile([P, 1], BF16)
            nc.gpsimd.memset(ones_col, 1.0)
            cos_sb = const_pool.tile([P, S], BF16)
            sin_sb = const_pool.tile([P, S], BF16)
            cos_srcs = (cosT.ap(), sinT.ap())

            # weight + first-strip pools live outside the loop so the
            # next iteration's loads never write into SBUF regions the
            # previous iteration's phase 3 still reads (their WAR
            # clears at the previous Q/K pass instead)
            with (
                tc.tile_pool(name="wqkv", bufs=1) as w_pool,
                tc.tile_pool(name="xpre", bufs=1) as xpre_pool,
            ):
                wq_sb = w_pool.tile([P, EC, NH_LOC * H], BF16)
                wk_sb = w_pool.tile([P, EC, NH_LOC * H], BF16)
                wv_sb = w_pool.tile([P, EC, NH_LOC * H], BF16)
                w_sbs = (wq_sb, wk_sb, wv_sb)

                loop_cm = tc.For_i(0, loop_trips, 1) if loop_trips else \
                    contextlib.nullcontext()
                with loop_cm:
                    _build_body(nc, tc, reps, with_cc, out, ag_in,
                                ag_out1, ag_out2, cos_sb, sin_sb,
                                cos_srcs, maskT, ones_col, w_sbs, xpre_pool,
                                xqT_r, xkT_r, wq_r, wk_r, wv_r, wo_r)

    nc.compile()
    return nc


def _build_body(nc, tc, reps, with_cc, out, ag_in, ag_out1, ag_out2,
                cos_sb, sin_sb, cos_srcs, maskT, ones_col, w_sbs, xpre_pool,
                xqT_r, xkT_r, wq_r, wk_r, wv_r, wo_r):
    for rep in range(reps):
        sfx = f"_r{rep}" if reps > 1 else ""
        with tc.tile_pool(name="qkv" + sfx, bufs=1) as qkv_pool:
            qT_sc = [qkv_pool.tile([P, NH_LOC, 512], BF16, name=f"qT{sc}")
                     for sc in range(4)]
            kT_sc = [qkv_pool.tile([P, NH_LOC, 512], BF16, name=f"kT{sc}")
                     for sc in range(4)]
            v_sc = [qkv_pool.tile([P, 4, NH_LOC * H], BF16, name=f"v{sc}")
                    for sc in range(4)]

            _phase1(nc, tc, qT_sc, kT_sc, v_sc, cos_sb, sin_sb, cos_srcs,
                    w_sbs, xpre_pool, xqT_r, xkT_r, wq_r, wk_r, wv_r, sfx)

            # prefetch WO for phase 3 (SBUF freed by phase-1 pools)
            with tc.tile_pool(name="wo_pool" + sfx, bufs=1) as wo_pool:
                wo_sb = wo_pool.tile([P, N, ECHUNK], BF16)
                nc.sync.dma_start(wo_sb[:], wo_r)

                with (
                    tc.tile_pool(name="elt" + sfx, bufs=3) as elt_pool,
                    tc.tile_pool(name="qsum" + sfx, bufs=2) as qsum_pool,
                    tc.tile_pool(name="rc" + sfx, bufs=2) as rc_pool,
                    tc.tile_pool(name="bc" + sfx, bufs=2) as bc_pool,
                    tc.tile_pool(name="at" + sfx, bufs=2) as at_pool,
                    # ao lives alongside the attention pools (disjoint
                    # SBUF) so the gather DMAs can land while attention
                    # is still running
                    tc.tile_pool(name="ao" + sfx, bufs=1) as ao_pool,
                ):
                    consts = (maskT, ones_col)
                    # two 512-col panes, refilled mid-phase-3, so ao fits
                    # alongside the hoisted weight pool
                    ao_sb = ao_pool.tile([P, N, 2 * 512], BF16)

                    def gather_ao(sch):
                        # contiguous 4KB-per-partition reads (p-major ag)
                        pane = (sch % 2) * 512
                        for r in range(4):
                            if sch < 3:
                                s3 = ag_out1.ap()[r, sch]
                            else:
                                s3 = ag_out2.ap()[r]
                            nc.sync.dma_start(
                                ao_sb[:, r * NH_LOC:(r + 1) * NH_LOC,
                                      pane:pane + 512],
                                s3)

                    with (
                        tc.tile_pool(name="lt_psum" + sfx, bufs=2,
                                     space="PSUM") as lt_psum,
                        tc.tile_pool(name="av_psum" + sfx, bufs=3,
                                     space="PSUM") as av_psum,
                        tc.tile_pool(name="sm_psum" + sfx, bufs=1,
                                     space="PSUM") as sm_psum,
                    ):
                        pools = (elt_pool, qsum_pool, rc_pool, bc_pool,
                                 at_pool, lt_psum, av_psum, sm_psum)
                        pending = []
                        # ACT-heavy sc2 chains interleave with ACT-light
                        # sc0/sc1 chains so the exp engine never bubbles;
                        # all sc<=2 chains still precede AG1
                        order = [(2, 0), (0, 0), (1, 0), (2, 1), (0, 1),
                                 (1, 1), (2, 2), (0, 2), (1, 2), (2, 3),
                                 (0, 3), (1, 3)]
                        for sc, n in order:
                            _attn_sc(nc, tc, pools, sc, [n], qT_sc,
                                     kT_sc, v_sc, consts, ag_in, pending)
                        # flush before AG1 so cols 0:1536 are final
                        while pending:
                            _flush_one(nc, pools, ag_in, pending)
                        if with_cc:
                            nc.gpsimd.collective_compute(
                                "AllGather", mybir.AluOpType.bypass,
                                replica_groups=REPLICA_GROUPS,
                                ins=[ag_in.ap()[0:3].opt()],
                                outs=[ag_out1.ap().opt()])
                        gather_ao(0)
                        gather_ao(1)
                        for n in range(NH_LOC):
                            _attn_sc(nc, tc, pools, 3, [n], qT_sc,
                                     kT_sc, v_sc, consts, ag_in, pending)
                        while pending:
                            _flush_one(nc, pools, ag_in, pending)
                        if with_cc:
                            nc.gpsimd.collective_compute(
                                "AllGather", mybir.AluOpType.bypass,
                                replica_groups=REPLICA_GROUPS,
                                ins=[ag_in.ap()[3].opt()],
                                outs=[ag_out2.ap().opt()])

                    # -------- phase 3: output projection --------
                    with (
                        tc.tile_pool(name="pso" + sfx, bufs=4,
                                     space="PSUM") as pso_pool,
                        tc.tile_pool(name="osb" + sfx, bufs=3) as o_pool,
                    ):
                        for st in range(ST):
                            ssl = slice(st * P, (st + 1) * P)
                            pane = ((st // 4) % 2) * 512
                            asl = slice(pane + (st % 4) * P,
                                        pane + (st % 4 + 1) * P)
                            pso = pso_pool.tile([P, ECHUNK], F32, tag="pso")
                            for n in range(N):
                                nc.tensor.matmul(
                                    pso, ao_sb[:, n, asl], wo_sb[:, n, :],
                                    start=(n == 0), stop=(n == N - 1))
                            osb = o_pool.tile([P, ECHUNK], F32, tag="osb")
                            nc.scalar.copy(out=osb[:], in_=pso[:])
                            nc.scalar.dma_start(out.ap()[ssl, :], osb[:])
                            if st == 3:
                                gather_ao(2)   # pane 0 refill
                            elif st == 7:
                                gather_ao(3)   # pane 1 refill


def _flush_one(nc, pools, ag_in, pending):
    pending.pop(0)()


_NC_CACHE = None


def _get_nc():
    global _NC_CACHE
    if _NC_CACHE is None:
        _NC_CACHE = _build()
    return _NC_CACHE


def _prepare_in_maps(x_q, x_kv, WQ, WK, WV, WO):
    bf = ml_dtypes.bfloat16
    cosT, sinT = _rope_tables()
    wo_flat = WO.reshape(N * H, E)

    in_maps = []
    xT_cache = {}
    for c in range(NCORES):
        b, hg = c // 4, c % 4
        hsl = slice(hg * NH_LOC, (hg + 1) * NH_LOC)
        esl = slice(hg * ECHUNK, (hg + 1) * ECHUNK)
        if b not in xT_cache:
            xT_cache[b] = (
                np.ascontiguousarray(x_q[b].T.astype(bf)),
                np.ascontiguousarray(x_kv[b].T.astype(bf)),
            )
        xqTb, xkTb = xT_cache[b]
        in_maps.append({
            "xqT": xqTb,
            "xkT": xkTb,
            "wq": np.ascontiguousarray(WQ[:, hsl, :].reshape(E, NH_LOC * H).astype(bf)),
            "wk": np.ascontiguousarray(WK[:, hsl, :].reshape(E, NH_LOC * H).astype(bf)),
            "wv": np.ascontiguousarray(WV[:, hsl, :].reshape(E, NH_LOC * H).astype(bf)),
            "wo": np.ascontiguousarray(wo_flat[:, esl].astype(bf)),
            "cosT": cosT,
            "sinT": sinT,
        })
    return in_maps


def kernel(x_q, x_kv, WQ, WK, WV, WO):
    global LAST_RESULTS
    in_maps = _prepare_in_maps(x_q, x_kv, WQ, WK, WV, WO)
    nc = _get_nc()
    res = run_bass_kernel_spmd(nc, in_maps, core_ids=list(range(NCORES)),
                               trace=TRACE)
    LAST_RESULTS = res

    out = np.empty((B, S, E), dtype=np.float32)
    for c in range(NCORES):
        b, j = c // 4, c % 4
        out[b, :, j * ECHUNK:(j + 1) * ECHUNK] = res.results[c]["out"]
    return out
